# revision 19
# baseline (speedup 1.0000x reference)
"""Trainium2 Bass kernel for per-expert 2-layer MLP (grouped GEMM -> GELU -> grouped GEMM).

reference: hidden = einsum('end,edh->enh', x, w1); gelu(erf); out = einsum('enh,ehd->end', h, w2)
shapes:    x [16, 2048, 1024] f32, w1 [16, 1024, 4096] f32, w2 [16, 4096, 1024] f32

Expert-parallel over 8 NeuronCores: core c owns experts [2c, 2c+1], no
cross-core communication.  Per core, per expert:

  phase A:  actT[h, n] = gelu(w1[d, h].T @ xT[d, n])   (PE matmul, contraction d)
  phase B:  out[n, d'] = actT[h, n].T @ w2[h, d']      (PE matmul, contraction h)

The fp16 PE roofline for this workload is 873.8 us/core.  To go below it,
1/4 of phase B's H-contraction (h-blocks 0..7) runs in fp8-e4m3 DoubleRow
mode (2 rows/cycle, 2x PE throughput).  e4m3 has only 3 mantissa bits, so
that slice is computed in a separate PSUM accumulator and the operands are
range-managed: w2 is pre-scaled by 2^13 on the host (unscaled, all of w2
sits in e4m3's denormal range and would quantize at ~12% error), and the
fp8 partial sum is rescaled by 2^-13 during the PSUM->SBUF copy (ScalarE
Copy-with-scale), then fused with the fp16 partial via a DVE add.
Measured end-to-end rel err 1.780e-2 (sim-verified 1.795e-2 under the
e4m3-denormal-flush worst case) vs the 2e-2 gate.  Everything else keeps
the PE gapless:

- All DRAM operands are pre-swizzled on the host so every DMA descriptor
  moves a >=2KB contiguous run on both the DRAM and SBUF side (the naive
  layouts produce 256B-1KB packets that are descriptor-rate bound).
- w1 arrives in 128-h-column chunks so phase A's h-block j only waits on
  chunk j; the first x block + first w1 chunk ride the sync queue, whose
  stream starts ~3us before the others.
- Zero-input warmup matmuls issue while the first loads are in flight and
  walk the PE out of its low DVFS p-state (634ns -> 380ns per 512-row
  matmul) so the first real chain starts at full clock.
- Phase B shares each stationary actT column-block between the two d'
  chunks; output stores are split per d'-half so the last store's DMA
  starts as early as possible.
Matmuls accumulate in fp32 PSUM; GELU (erf) runs on ScalarE out of PSUM.
"""

import os
import sys

import ml_dtypes
import numpy as np

for _p in ("/opt/trn_rl_repo", "/root/.axon_site/_ro/trn_rl_repo"):
    if os.path.isdir(_p) and _p not in sys.path:
        sys.path.append(_p)

import concourse.bacc as bacc
import concourse.tile as tile
from concourse import mybir
from concourse.bass_utils import run_bass_kernel_spmd

E, N, D, H = 16, 2048, 1024, 4096
NCORES = 8
EPC = E // NCORES        # experts per core                     = 2
P = 128                  # SBUF partitions
FD = 512                 # matmul moving free dim
NB = 512                 # token block per phase-A/B iteration
N_BLOCKS = N // NB       # = 4
N_SUB = NB // P          # row sub-blocks per token block       = 4
KD = D // P              # d-blocks (contraction of matmul 1)   = 8
KH = H // P              # h-blocks (contraction of matmul 2)   = 32
KH8 = 8                  # h-blocks 0..7 of matmul 2 run in fp8 DoubleRow
KH16 = KH - KH8          # h-blocks KH8..31 run in fp16
DC = D // FD             # d' chunks (free dim of matmul 2)     = 2
SW_LOG2 = 13             # host pre-scale of fp8 w2 slice (2^13)
F16 = mybir.dt.float16
F32 = mybir.dt.float32
F8 = mybir.dt.float8e4
N_WARMUP = 17            # zero matmuls to ramp the PE p-state

_CACHE = {}


def _build():
    nc = bacc.Bacc(None, target_bir_lowering=False)
    # host-swizzled layouts (see _prep): every DMA is contiguous-run friendly
    xb_d = nc.declare_dram_parameter("xb", [EPC, N_BLOCKS, P, KD, NB], F16, isOutput=False)
    w1_d = nc.declare_dram_parameter("w1b", [EPC, KH, P, KD, P], F16, isOutput=False)
    w2_d = nc.declare_dram_parameter("w2b", [EPC, P, KH16, D], F16, isOutput=False)
    w28_d = nc.declare_dram_parameter("w28b", [EPC, P, KH8 // 2, 2, D], F8, isOutput=False)
    out_d = nc.declare_dram_parameter("out", [EPC, N, D], F32, isOutput=True)

    with (
        tile.TileContext(nc) as tc,
        tc.tile_pool(name="warm", bufs=1) as warm_pool,
        tc.tile_pool(name="w1sb", bufs=1) as w1_pool,
        tc.tile_pool(name="w2sb", bufs=1) as w2_pool,
        tc.tile_pool(name="w28sb", bufs=1) as w28_pool,
        tc.tile_pool(name="xT", bufs=2) as xt_pool,
        tc.tile_pool(name="actT", bufs=1) as act_pool,
        tc.tile_pool(name="actT8", bufs=1) as act8_pool,
        tc.tile_pool(name="osb", bufs=3) as out_pool,
        tc.tile_pool(name="ps_1", bufs=4, space="PSUM") as ps1_pool,
        tc.tile_pool(name="ps_2", bufs=4, space="PSUM") as ps2_pool,
    ):

        def emit_warmup():
            """Zero matmuls with no DMA dependency: they start right after
            the prologue barrier while the first loads are still in flight
            and walk the PE up to its max p-state (~3us of busy time)."""
            wz = warm_pool.tile([P, NB], F16, tag="wz")
            nc.vector.memset(wz, 0.0)
            for _ in range(N_WARMUP):
                pw = ps1_pool.tile([P, NB], F32, tag="ps1")
                nc.tensor.matmul(pw, lhsT=wz[:, 0:P], rhs=wz, start=True, stop=True)

        def emit_x_loads(e, nb):
            """one 8KB-run DMA per 512-token block."""
            xt_sb = xt_pool.tile([P, KD, NB], F16, tag="xT")
            nc.sync.dma_start(out=xt_sb, in_=xb_d[e, nb])
            return xt_sb

        def emit_w1_loads(e):
            """w1 for expert e: SBUF [p, hc, k, hj] (hc = 32 chunks of 128 h).
            DRAM layout [e, hc, p, k, hj] makes each chunk a 2KB-run DMA.
            First 4 chunks go as single-chunk DMAs so phase A's first
            h-blocks unblock ASAP; the rest batch 4 chunks per trigger.
            Chunk 0 of expert 0 rides the sync queue (early stream), right
            behind the first x block."""
            w1_sb = w1_pool.tile([P, KH, KD, P], F16, tag="w1")
            w1_view = w1_d[e].rearrange("c p k h -> p c k h")
            eng = nc.sync if e == 0 else nc.gpsimd
            eng.dma_start(out=w1_sb[:, 0:1], in_=w1_view[:, 0:1])
            for hc in range(1, 4):
                nc.gpsimd.dma_start(
                    out=w1_sb[:, hc : hc + 1], in_=w1_view[:, hc : hc + 1]
                )
            for c in range(1, 8):
                nc.gpsimd.dma_start(
                    out=w1_sb[:, 4 * c : 4 * c + 4], in_=w1_view[:, 4 * c : 4 * c + 4]
                )
            return w1_sb

        def emit_w2_loads(e):
            """whole-expert w2 (fp16 h-blocks KH8..31 + fp8 h-blocks 0..KH8-1):
            DRAM [e, p, ...] layouts give 4-64KB runs/partition."""
            w28_sb = w28_pool.tile([P, KH8 // 2, 2, D], F8, tag="w28")
            nc.gpsimd.dma_start(out=w28_sb, in_=w28_d[e])
            w2_sb = w2_pool.tile([P, KH16, D], F16, tag="w2")
            hb = KH16 // 2
            for c in range(2):
                nc.gpsimd.dma_start(
                    out=w2_sb[:, c * hb : (c + 1) * hb],
                    in_=w2_d[e][:, c * hb : (c + 1) * hb],
                )
            return w2_sb, w28_sb

        def emit_phase_a(w1_sb, xt_sb):
            actT = act_pool.tile([P, KH16, NB], F16, tag="actT")
            actT8 = act8_pool.tile([P, KH8 // 2, 2, NB], F8, tag="actT8")
            for h in range(KH):
                ps1 = ps1_pool.tile([P, NB], F32, tag="ps1")
                for k in range(KD):
                    nc.tensor.matmul(
                        ps1,
                        lhsT=w1_sb[:, h, k],
                        rhs=xt_sb[:, k],
                        start=(k == 0),
                        stop=(k == KD - 1),
                    )
                dst = actT8[:, h // 2, h % 2] if h < KH8 else actT[:, h - KH8]
                nc.scalar.activation(dst, ps1, mybir.ActivationFunctionType.Gelu)
            return actT, actT8

        def emit_phase_b(e, nb, actT, actT8, w2_sb, w28_sb):
            n0 = nb * NB
            inv = float(2.0 ** (-SW_LOG2))
            last_block = e == EPC - 1 and nb == N_BLOCKS - 1
            for s in range(N_SUB):
                osb = out_pool.tile([P, D], F32, tag="osb")
                rows = out_d[e, n0 + s * P : n0 + (s + 1) * P, :]
                p8a = ps1_pool.tile([P, FD], F32, tag="ps1")
                p8b = ps1_pool.tile([P, FD], F32, tag="ps1")
                pa = ps2_pool.tile([P, FD], F32, tag="ps2")
                pb = ps2_pool.tile([P, FD], F32, tag="ps2")
                NT8 = KH8 // 2

                def dr_step(t):
                    # fp8 slice: h-blocks 0..KH8-1 as DoubleRow pairs into
                    # their own accumulators (carrying the 2^13 w2 pre-scale)
                    l8 = actT8[:, t // 2, :, s * P : (s + 1) * P]
                    tgt, w8 = (p8a, w28_sb[:, t // 2, :, 0:FD]) if t % 2 == 0 \
                        else (p8b, w28_sb[:, t // 2, :, FD:D])
                    nc.tensor.matmul(tgt, lhsT=l8, rhs=w8,
                                     start=(t < 2), stop=(t >= 2 * NT8 - 2),
                                     perf_mode=mybir.MatmulPerfMode.DoubleRow)

                def f16_step(h, which):
                    lhsT = actT[:, h, s * P : (s + 1) * P]
                    tgt, w = (pa, w2_sb[:, h, 0:FD]) if which == 0 \
                        else (pb, w2_sb[:, h, FD:D])
                    nc.tensor.matmul(tgt, lhsT=lhsT, rhs=w,
                                     start=(h == 0), stop=(h == KH16 - 1))

                if last_block and s == N_SUB - 1:
                    # drain-friendly tail: finish the c0 half completely
                    # first so its store overlaps the c1 chain and the final
                    # store's critical path is just add+DMA of one half.
                    for t in range(0, 2 * NT8, 2):
                        dr_step(t)
                    nc.scalar.activation(osb[:, 0:FD], p8a,
                                         mybir.ActivationFunctionType.Copy, scale=inv)
                    for h in range(KH16):
                        f16_step(h, 0)
                    nc.vector.tensor_tensor(osb[:, 0:FD], osb[:, 0:FD], pa,
                                            mybir.AluOpType.add)
                    nc.sync.dma_start(out=rows[:, 0:FD], in_=osb[:, 0:FD])
                    for t in range(1, 2 * NT8, 2):
                        dr_step(t)
                    nc.scalar.activation(osb[:, FD:D], p8b,
                                         mybir.ActivationFunctionType.Copy, scale=inv)
                    for h in range(KH16):
                        f16_step(h, 1)
                    # final store: split the add and fan the two quarter
                    # stores over two queue engines so descriptor processing
                    # of the kernel's last bytes runs in parallel.
                    HF = FD // 2
                    nc.vector.tensor_tensor(osb[:, FD:FD + HF], osb[:, FD:FD + HF],
                                            pb[:, 0:HF], mybir.AluOpType.add)
                    nc.scalar.dma_start(out=rows[:, FD:FD + HF], in_=osb[:, FD:FD + HF])
                    nc.vector.tensor_tensor(osb[:, FD + HF:D], osb[:, FD + HF:D],
                                            pb[:, HF:FD], mybir.AluOpType.add)
                    nc.sync.dma_start(out=rows[:, FD + HF:D], in_=osb[:, FD + HF:D])
                    continue
                for t in range(0, 2 * NT8, 2):
                    dr_step(t)
                for t in range(1, 2 * NT8, 2):
                    dr_step(t)
                for h in range(KH16):
                    f16_step(h, 0)
                    f16_step(h, 1)
                nc.scalar.activation(osb[:, 0:FD], p8a,
                                     mybir.ActivationFunctionType.Copy, scale=inv)
                nc.scalar.activation(osb[:, FD:D], p8b,
                                     mybir.ActivationFunctionType.Copy, scale=inv)
                nc.vector.tensor_tensor(osb[:, 0:FD], osb[:, 0:FD], pa,
                                        mybir.AluOpType.add)
                nc.sync.dma_start(out=rows[:, 0:FD], in_=osb[:, 0:FD])
                nc.vector.tensor_tensor(osb[:, FD:D], osb[:, FD:D], pb,
                                        mybir.AluOpType.add)
                nc.sync.dma_start(out=rows[:, FD:D], in_=osb[:, FD:D])

        emit_warmup()
        w1_cur = emit_w1_loads(0)  # w1 chunk 0 leads the sync queue
        xt_first = emit_x_loads(0, 0)
        w1_next = None
        w2_cur = w28_cur = None
        for e in range(EPC):
            for nb in range(N_BLOCKS):
                xt_sb = xt_first if (e, nb) == (0, 0) else emit_x_loads(e, nb)
                actT, actT8 = emit_phase_a(w1_cur, xt_sb)
                if nb == 0:
                    if e == 0:
                        # Stall the w2 slot until phase A is underway: its 8MB
                        # stream otherwise competes with the startup-critical
                        # w1 chunk + x block loads for the HBM window.
                        gate = w2_pool.tile([P, 4], F32, tag="w2")
                        nc.vector.tensor_copy(gate, actT[:, 4, 0:4])
                    w2_cur, w28_cur = emit_w2_loads(e)
                if nb == N_BLOCKS - 1 and e + 1 < EPC:
                    w1_next = emit_w1_loads(e + 1)
                emit_phase_b(e, nb, actT, actT8, w2_cur, w28_cur)
            w1_cur = w1_next

    nc.compile()
    return nc


def _get_nc():
    if "nc" not in _CACHE:
        _CACHE["nc"] = _build()
    return _CACHE["nc"]


def _prep(inputs):
    x = np.asarray(inputs["x"], dtype=np.float32).astype(np.float16)
    w1 = np.asarray(inputs["w1"], dtype=np.float32).astype(np.float16)
    w2f = np.asarray(inputs["w2"], dtype=np.float32)
    w2 = w2f.astype(np.float16)
    # xb[e, nb, p, k, nj] = x[e, nb*512+nj, k*128+p]
    xb = np.ascontiguousarray(
        x.reshape(E, N_BLOCKS, NB, KD, P).transpose(0, 1, 4, 3, 2)
    )
    # w1b[e, hc, p, k, hj] = w1[e, k*128+p, hc*128+hj]
    w1b = np.ascontiguousarray(
        w1.reshape(E, KD, P, KH, P).transpose(0, 3, 2, 1, 4)
    )
    # w2b[e, p, hb, d] = w2[e, (hb+KH8)*128+p, d]   (fp16 h-blocks KH8..31)
    w2r = w2.reshape(E, KH, P, D)
    w2b = np.ascontiguousarray(w2r[:, KH8:].transpose(0, 2, 1, 3))
    # w28b[e, p, t, j, d] = e4m3(w2[e, (2t+j)*128+p, d] * 2^13)
    w2s = w2f.reshape(E, KH, P, D)[:, :KH8].transpose(0, 2, 1, 3) * (2.0 ** SW_LOG2)
    assert np.abs(w2s).max() < 224.0, "fp8 w2 slice overflows e4m3 range"
    w28b = np.ascontiguousarray(
        w2s.reshape(E, P, KH8 // 2, 2, D).astype(ml_dtypes.float8_e4m3)
    )
    return xb, w1b, w2b, w28b


def _run(inputs, trace=False, trace_cores=None):
    xb, w1b, w2b, w28b = _prep(inputs)
    nc = _get_nc()
    in_maps = [
        {
            "xb": xb[c * EPC : (c + 1) * EPC],
            "w1b": w1b[c * EPC : (c + 1) * EPC],
            "w2b": w2b[c * EPC : (c + 1) * EPC],
            "w28b": w28b[c * EPC : (c + 1) * EPC],
        }
        for c in range(NCORES)
    ]
    res = run_bass_kernel_spmd(
        nc, in_maps, list(range(NCORES)), trace=trace, trace_cores=trace_cores
    )
    out = np.concatenate([res.results[c]["out"] for c in range(NCORES)], axis=0)
    return out.astype(np.float32, copy=False), res


def kernel(**inputs) -> np.ndarray:
    out, _ = _run(inputs, trace=False)
    return out
